# revision 2
# baseline (speedup 1.0000x reference)
"""Grouped-channel attention (CAT FullAttention) Trainium2 kernel, v2.

Math (per batch element b; L=S=96, R=70, E=10, P=7):
  scores[l,s,p,r] = sum_e q[l,e,p] * k[s,e,r]
  A = softmax over (s,p) of scores           (per l, r)
  out[l,e,r]      = sum_{s,p} v[s,e,p] * A[l,s,p,r]

v2 packs score windows to 128 partitions: the e1 weight matrix is the
concatenated K' [e, r*96+s] sliced at 128-column boundaries (legal since
every column shares the same contraction (e) and the same moving Q), so
each batch needs 5 full [128, 672] windows plus a 32-partition tail; the
four tails of a DMA group share one window (4*32 = 128). Exp columns per
batch drop 4704 -> 3528 (21 windows per 4-batch group), which is the
ACT/DVE bottleneck. Window exp assignment: 12 ACT (true exp) / 9 DVE
(Schraudolph) per group, interleaved.

e2 consumes A r-slices at their packed partition offsets {0,96,64,32};
slices straddling a window boundary split into two accumulation matmuls
(11 pieces x 7 p = 77 matmuls of 11 rows per batch).

qk loads as [40, 1344] (4 batches x E partitions) instead of [10, 4*1344]
for 4x DMA descriptor throughput.

Pipeline per group g: tail(g) windows first, then b0..b3 full windows;
e2(b) lags one batch (runs during b+1's windows); stage3 + grouped
output flush as before.
"""

import math
import sys

if "/opt/trn_rl_repo" not in sys.path:
    sys.path.insert(0, "/opt/trn_rl_repo")

import numpy as np
import ml_dtypes

import concourse.bass as bass
import concourse.bacc as bacc
import concourse.tile as tile
from concourse import mybir
from concourse.bass_utils import run_bass_kernel_spmd

B, L, R = 256, 96, 70
E, P = 10, 7
EP = E + 1  # v channels + ones channel
NCORES = 8
BPC = B // NCORES  # batches per core
G = 4  # batches per DMA group == supergroup
F32 = mybir.dt.float32
F16 = mybir.dt.float16
BF16 = mybir.dt.bfloat16
I16 = mybir.dt.int16

A16 = 128.0 / math.log(2.0)  # Schraudolph scale, folded into Q on host
B16 = 16256.0 - 6.0  # 127<<7 minus calibrated bias C=6

WFULL = 5  # full 128-part windows per batch (5*128 = 640 of 672 K' cols)
TAILC = 672 - WFULL * 128  # 32 K' cols in the shared tail window
NWIN = G * WFULL + 1  # windows per group incl shared tail

# Exp engine = global window parity: NWIN=21 is odd, so the A/D
# alternation runs seamlessly across group boundaries. Balance comes
# from stage3's copy running on ACT.

# e2 chains of the previous batch interleave with the producer windows:
# after window w of batch b, emit these r-chains of batch b-1.
WSPLIT = {0: (0, 1), 1: (2,), 2: (3, 4), 3: (5,), 4: (6,)}

# V replicas: VR_k[q] = v[q - SHIFTS[k]] inside the shifted range and 0
# outside. Every e2 matmul contracts the FULL 128 window partitions at
# base 0 / tile_position (0,0) — rows outside a piece's s-range hit the
# replica's zero padding, so other tenants of the window contribute
# nothing. This keeps every PSUM accumulation chain at one uniform
# tile config, which HW requires (mixed-position chains corrupt PSUM,
# found by probing). Replicas 6..8 are tail-only: v[s=64..96] placed at
# rows 32*bi for bi=1..3 (bi=0 coincides with shift -64).
SHIFTS = (0, 32, 64, 96, -32, -64)
NSH = 9

# e2 chains per r: list of (window index or 'T', replica k); each chain
# accumulates 7p x pieces matmuls of [128 contraction, 11 cols].
E2_CHAINS = {
    0: [(0, 0)],
    1: [(0, 3), (1, 4)],
    2: [(1, 2), (2, 5)],
    3: [(2, 1)],
    4: [(3, 0)],
    5: [(3, 3), (4, 4)],
    6: [(4, 2), ("T", -1)],  # tail k = TAILK[bi]
}
TAILK = (5, 6, 7, 8)

_CACHE = {}


def _build(bpc, dbg_a=False):
    nc = bacc.Bacc("TRN2", target_bir_lowering=False, debug=False, num_devices=NCORES)
    if dbg_a:
        a_dbg = nc.dram_tensor(
            "a_dbg", [bpc, 128, WFULL * 672], BF16, kind="ExternalOutput"
        ).ap()
    qk_d = nc.dram_tensor("qk", [bpc, E, 2 * P * L], F16, kind="ExternalInput").ap()
    v_d = nc.dram_tensor(
        "vt", [bpc, 128, NSH * P * EP], BF16, kind="ExternalInput"
    ).ap()
    # raw numerator/denominator regions; host does the divide
    o_d = nc.dram_tensor("out", [bpc, L, P * EP], F32, kind="ExternalOutput").ap()

    ngroups = bpc // G

    with tile.TileContext(nc) as tc:
        with (
            tc.tile_pool(name="const", bufs=1) as cpool,
            tc.tile_pool(name="qk", bufs=6) as qkpool,
            tc.tile_pool(name="aw", bufs=5) as awpool,
            tc.tile_pool(name="at", bufs=3) as atpool,
            tc.tile_pool(name="og", bufs=3) as ogpool,
            tc.tile_pool(name="ps", bufs=1, space="PSUM") as pspool,
        ):
            # 8 PSUM banks, manual layout: 4 rotating score windows at
            # 1024-col alignment (cols 0..672 of each slot used); e2
            # accumulators [96, 77] live in the slot-0 / slot-2 gaps
            # (cols 672..749 and 2720..2797), batch-parity interleaved.
            PS = pspool.tile([128, 4096], F32, tag="PS", name="PS")

            # touch Exp once so the ACT table set loads during the first DMA
            warm = cpool.tile([1, 2], F32)
            nc.vector.memset(warm[:, 0:1], 0.0)
            nc.scalar.activation(
                warm[:, 1:2], warm[:, 0:1], mybir.ActivationFunctionType.Exp
            )

            qg = [None] * ngroups
            vg = [None] * ngroups
            AW = [None] * bpc  # per-batch [128, 5*672] bf16 exp windows
            AT = [None] * ngroups  # per-group shared tail [128, 672]
            OG = [None] * ngroups
            gw = [0]  # global window counter (rotation slot = gw % 4)

            def load_group(g):
                # batch bi lives at partitions 32*bi..32*bi+10 (quadrant-
                # aligned bases for PE tile_position)
                qkt = qkpool.tile([128, 2 * P * L], F16, tag="qkg")
                vt = qkpool.tile([128, G * NSH * P * EP], BF16, tag="vg")
                g0 = g * G
                for bi in range(G):
                    nc.sync.dma_start(
                        qkt[32 * bi : 32 * bi + E, :], qk_d[g0 + bi]
                    )
                nc.sync.dma_start(
                    vt[:].rearrange("q (b f) -> q b f", b=G),
                    v_d[g0 : g0 + G].rearrange("b q f -> q b f"),
                )
                qg[g], vg[g] = qkt, vt

            def exp_window(src, dst_ap, dst_i16, gwi):
                """Dispatch exp of one PSUM window by global parity."""
                if gwi % 2 == 0:
                    nc.scalar.activation(
                        dst_ap,
                        src,
                        mybir.ActivationFunctionType.Exp,
                        scale=1.0 / A16,
                    )
                else:
                    nc.vector.tensor_scalar_add(dst_i16, src, B16)

            def e1_full(b, bi, w):
                """One full [128, 672] score window for batch b, K'-slice w."""
                g = b // G
                qkt = qg[g]
                p0 = 32 * bi
                if w == 0:
                    AW[b] = awpool.tile(
                        [128, WFULL * 672], BF16, tag="AW", name=f"AW{b}"
                    )
                gwi = gw[0]
                off = (gwi % 4) * 1024
                gw[0] += 1
                kof = 672 + 128 * w
                lhs = qkt[p0 : p0 + E, kof : kof + 128]
                nc.tensor.matmul(
                    PS[:, off : off + 512],
                    lhsT=lhs,
                    rhs=qkt[p0 : p0 + E, 0:512],
                    start=True,
                    stop=True,
                    tile_position=(p0, 0),
                )
                nc.tensor.matmul(
                    PS[:, off + 512 : off + 672],
                    lhsT=lhs,
                    rhs=qkt[p0 : p0 + E, 512:672],
                    start=True,
                    stop=True,
                    tile_position=(p0, 0),
                )
                cols = slice(w * 672, (w + 1) * 672)
                exp_window(
                    PS[:, off : off + 672],
                    AW[b][:, cols],
                    AW[b][:, cols].bitcast(I16),
                    gwi,
                )

            def e1_tail(g):
                """Shared tail window: batch bi's K' cols 640..672 land at
                partitions 32*bi..32*bi+32."""
                qkt = qg[g]
                AT[g] = atpool.tile([128, 672], BF16, tag="AT", name=f"AT{g}")
                gwi = gw[0]
                off = (gwi % 4) * 1024
                gw[0] += 1
                for bi in range(G):
                    p0 = 32 * bi
                    lhs = qkt[p0 : p0 + E, 672 + 640 : 672 + 640 + TAILC]
                    nc.tensor.matmul(
                        PS[32 * bi : 32 * bi + TAILC, off : off + 512],
                        lhsT=lhs,
                        rhs=qkt[p0 : p0 + E, 0:512],
                        start=True,
                        stop=True,
                        tile_position=(p0, 32 * bi),
                    )
                    nc.tensor.matmul(
                        PS[32 * bi : 32 * bi + TAILC, off + 512 : off + 672],
                        lhsT=lhs,
                        rhs=qkt[p0 : p0 + E, 512:672],
                        start=True,
                        stop=True,
                        tile_position=(p0, 32 * bi),
                    )
                exp_window(
                    PS[:, off : off + 672],
                    AT[g][:],
                    AT[g][:].bitcast(I16),
                    gwi,
                )

            def e2_chains(b, rs):
                """Uniform (0,0) full-contraction chains for r in rs."""
                g, bi = divmod(b, G)
                if dbg_a and rs and rs[0] == 0:
                    nc.gpsimd.dma_start(a_dbg[b], AW[b][:])
                vt = vg[g]
                base = 672 + (b % 2) * 2048
                vb = bi * NSH * P * EP
                for r in rs:
                    pieces = E2_CHAINS[r]
                    np_ = len(pieces)
                    dst = PS[0:L, base + r * EP : base + (r + 1) * EP]
                    for j, (wsel, k) in enumerate(pieces):
                        if wsel == "T":
                            k = TAILK[bi]
                        for p in range(P):
                            if wsel == "T":
                                asl = AT[g][:, p * L : (p + 1) * L]
                            else:
                                asl = AW[b][
                                    :, wsel * 672 + p * L : wsel * 672 + (p + 1) * L
                                ]
                            vcol = vb + k * P * EP + p * EP
                            nc.tensor.matmul(
                                dst,
                                lhsT=asl,
                                rhs=vt[:, vcol : vcol + EP],
                                start=(j == 0 and p == 0),
                                stop=(j == np_ - 1 and p == P - 1),
                                tile_position=(0, 0),
                            )
            OW = P * EP  # 77 raw output cols per batch

            def stage3(b):
                """Copy the raw accumulation regions PSUM -> SBUF staging."""
                g, bi = divmod(b, G)
                base = 672 + (b % 2) * 2048
                if OG[g] is None:
                    OG[g] = ogpool.tile([L, G * OW], F32, tag="og", name=f"og{g}")
                nc.scalar.copy(
                    OG[g][:, bi * OW : (bi + 1) * OW], PS[0:L, base : base + OW]
                )

            def flush_group(g, last=False):
                g0 = g * G
                ow = P * EP
                if last:
                    nc.gpsimd.dma_start(
                        o_d[g0 : g0 + G - 1].rearrange("b l c -> l b c"),
                        OG[g][:, 0 : (G - 1) * ow].rearrange(
                            "l (b c) -> l b c", b=G - 1
                        ),
                    )
                    nc.gpsimd.dma_start(
                        o_d[g0 + G - 1], OG[g][:, (G - 1) * ow : G * ow]
                    )
                else:
                    nc.gpsimd.dma_start(
                        o_d[g0 : g0 + G].rearrange("b l c -> l b c"),
                        OG[g][:].rearrange("l (b c) -> l b c", b=G),
                    )
                OG[g] = None

            # ---- pipeline ----
            # group g: tail(g) first, then b0..b3 windows; e2(b) lags one
            # batch, chains interleaved after producer windows per WSPLIT.
            load_group(0)
            for g in range(ngroups):
                if g + 1 < ngroups:
                    load_group(g + 1)
                e1_tail(g)
                for bi in range(G):
                    b = g * G + bi
                    for w in range(WFULL):
                        e1_full(b, bi, w)
                        if b > 0:
                            e2_chains(b - 1, WSPLIT[w])
                            if w == WFULL - 1:
                                stage3(b - 1)
                                if (b - 1) % G == G - 1:
                                    flush_group((b - 1) // G)
            # epilogue: last batch
            b = bpc - 1
            e2_chains(b, tuple(range(P)))
            stage3(b)
            flush_group(ngroups - 1, last=True)

    nc.compile()
    return nc


def _get_nc(bpc=BPC):
    if bpc not in _CACHE:
        _CACHE[bpc] = _build(bpc)
    return _CACHE[bpc]


def _prep(queries, keys, values):
    q = np.asarray(queries, dtype=np.float32)
    k = np.asarray(keys, dtype=np.float32)
    v = np.asarray(values, dtype=np.float32)
    b = q.shape[0]
    # QK[b, e, 0:672]   = A16 * q[b, l, e*7+p]  (p-major then l)
    # QK[b, e, 672:1344] = k[b, s, e*7+r]        (r-major then s)
    q2 = (q.reshape(b, L, E, P) * A16).transpose(0, 2, 3, 1).reshape(b, E, P * L)
    kt = k.reshape(b, L, E, P).transpose(0, 2, 3, 1).reshape(b, E, P * L)
    qk = np.ascontiguousarray(np.concatenate([q2, kt], axis=2)).astype(np.float16)
    # VT base [b, s, p*11+e'] = v[b, s, e'*7+p] for e'<10, 1.0 at e'=10;
    # replicated at partition shifts SHIFTS so e2 matmul lhsT/rhs share a
    # partition base: VR[b, q, k, :] = VT[b, q - SHIFTS[k], :]
    v4 = v.reshape(b, L, E, P).transpose(0, 1, 3, 2)  # [b, s, p, e]
    vtb = np.concatenate([v4, np.ones((b, L, P, 1), np.float32)], axis=-1)
    vtb = vtb.reshape(b, L, P * EP)
    vr = np.zeros((b, 128, NSH, P * EP), np.float32)
    for k, sh in enumerate(SHIFTS):
        q0, q1 = max(0, sh), min(128, L + sh)
        vr[:, q0:q1, k, :] = vtb[:, q0 - sh : q1 - sh, :]
    # tail replicas (k = 6, 7, 8): v[s=64..96] at rows 32*bi, bi = 1..3
    for bi in range(1, G):
        vr[:, 32 * bi : 32 * bi + 32, 5 + bi, :] = vtb[:, 64:96, :]
    vt = np.ascontiguousarray(vr.reshape(b, 128, NSH * P * EP)).astype(
        ml_dtypes.bfloat16
    )
    return qk, vt


def kernel(queries, keys, values, attn_mask=None, _trace=False):
    nc = _get_nc()
    qk, vt = _prep(queries, keys, values)
    in_maps = []
    for c in range(NCORES):
        s = slice(c * BPC, (c + 1) * BPC)
        in_maps.append({"qk": qk[s], "vt": vt[s]})
    res = None
    for attempt in range(3):
        try:
            res = run_bass_kernel_spmd(
                nc, in_maps, core_ids=list(range(NCORES)), trace=_trace
            )
            break
        except Exception:
            # shared terminal occasionally reports transient NRT device
            # errors; back off and retry
            if attempt == 2:
                raise
            import time as _time

            _time.sleep(15)
    out = np.concatenate([res.results[c]["out"] for c in range(NCORES)], axis=0)
    if _trace:
        kernel.last_exec_time_ns = res.exec_time_ns
        kernel.last_results = res
    return _postprocess(out)


def _postprocess(raw):
    """raw [B, L, 77] = [l, (r, e')]: divide by the ones-channel
    denominator, transpose to [l, (e, r)]."""
    b = raw.shape[0]
    raw = raw.astype(np.float32).reshape(b, L, P, EP)
    outr = raw[:, :, :, :E] / raw[:, :, :, E : E + 1]  # [b, l, r, e]
    return np.ascontiguousarray(
        outr.transpose(0, 1, 3, 2).reshape(b, L, R)
    )
